# revision 35
# baseline (speedup 1.0000x reference)
"""Trainium2 Bass kernel for nn_EnhancedEdgeScorer (gnn_message_passing).

Sharding: data-parallel over nodes (2048/core) and edges (8192/core) on 8
NeuronCores.  Per layer, each core computes K/V for its node shard, the
shards are AllGathered, and each core gathers its nodes' neighbor K/V rows
with dma_gather.  Key algebraic folds:
  - k/v are projected BEFORE the neighbor gather (gather commutes with the
    row-linear projection), turning the reference's (N*M,H)@(H,H) matmuls
    into (N,H)@(H,H).
  - k-bias drops out (softmax shift invariance); v-bias folds into the
    out-projection bias; the 1/sqrt(dh) scale folds into wq/bq.
Everything dense runs on the PE in bf16 with fp32 PSUM accumulation.
"""

import numpy as np
import ml_dtypes
from contextlib import ExitStack

import concourse.bass as bass
from concourse import bacc
import concourse.tile as tile
import concourse.mybir as mybir
from concourse.masks import make_identity
from concourse.bass_utils import run_bass_kernel_spmd

BF16 = mybir.dt.bfloat16
F32 = mybir.dt.float32
I16 = mybir.dt.int16

N, M, H, HEADS, L, E = 16384, 32, 256, 4, 3, 65536
DH = H // HEADS
T, V, CD = 8, 17, 64
TOTAL = H // 2 + 2 * CD + H // 4  # 320
NC = 8
NL = N // NC      # 2048 nodes per core
EL = E // NC      # 8192 edges per core
P = 128
NT = NL // P      # 16 node tiles per core
ET = EL // 512    # 16 edge chunks per core
NEG = -30.0       # additive pad-mask value (exp(-30) ~ 1e-13)

_bf = lambda a: np.ascontiguousarray(a.astype(ml_dtypes.bfloat16))
_f32 = lambda a: np.ascontiguousarray(a.astype(np.float32))


def _wrap16(idx):
    """Flat index list -> [128, len/16] int16 layout dma_gather expects
    (the 16-partition block is replicated for each of the 8 Q7 cores)."""
    idx = np.asarray(idx, dtype=np.int16)
    assert idx.size % 16 == 0
    return np.ascontiguousarray(np.tile(idx.reshape(-1, 16).T, (8, 1)))


# --------------------------------------------------------------------------
# Bass program (SPMD; per-core differences enter only through input data)
# --------------------------------------------------------------------------

def _mk_groups(m_ts):
    """Greedy-pack consecutive node tiles into gather groups of <= M slots,
    in reverse processing order (small tiles first, tile 0 last) so the
    per-layer tail is one short chain and each node-quarter's AllGather can
    fire as soon as its four tiles are projected."""
    groups, cur, s = [], [], 0
    for t in reversed(range(NT)):
        if cur and s + m_ts[t] > M:
            groups.append(cur)
            cur, s = [], 0
        cur.append(t)
        s += m_ts[t]
    groups.append(cur)
    return groups


def build_program(m_ts):
    """m_ts[t] = number of neighbor slots gathered for node tile t (<= M).
    Nodes are host-side sorted by context length (striped across cores) so
    m_ts is uniform across cores; padded slots beyond a node's length carry
    arbitrary valid indices and are masked in the softmax."""
    m_ts = list(m_ts)
    groups_g = _mk_groups(m_ts)
    goff = np.concatenate([[0], np.cumsum([sum(m_ts[t] for t in g) for g in groups_g])])
    IKV_W = int(goff[-1]) * P // 16  # idx words per layer (same list every layer)

    nc = bacc.Bacc(num_devices=NC)

    dp = lambda nm, shp, dt: nc.declare_dram_parameter(nm, list(shp), dt, isOutput=False)

    # ---- weights (same on all cores) ----
    type_tab = dp("type_tab", [T, H // 2], BF16)          # gather-T, elem 128
    cat_tab = dp("cat_tab", [V * V, 2 * CD], BF16)        # combined cat embeds
    dw = dp("dw", [1, H // 4], F32)                       # degree_w row
    db = dp("db", [H // 4], F32)
    projWT = dp("projWT", [3, P, H], BF16)                # proj_w.T in 3 row-chunks (zero padded)
    proj_b = dp("proj_b", [H], F32)
    wqT = dp("wqT", [L, 2, P, H], BF16)                   # (wq*scale).T row-chunks
    bq = dp("bq", [L, H], F32)                            # bq*scale
    wkT = dp("wkT", [L, 2, P, H], BF16)
    wvT = dp("wvT", [L, 2, P, H], BF16)
    woT = dp("woT", [L, 2, P, H], BF16)
    bo = dp("bo", [L, H], F32)                            # out_b + out_w@bv
    w1T = dp("w1T", [4, P, H], BF16)                  # mlp_w1.T eu/ev row-chunks
    w1eT = dp("w1eT", [2, H], BF16)                   # mlp_w1.T edge-feat rows
    b1 = dp("b1", [P, 2], F32)                            # b1 as [128, chunk]
    w2T = dp("w2T", [2, P, H // 2], BF16)
    b2 = dp("b2", [H // 2], F32)
    w3T = dp("w3T", [P, 1], BF16)
    b3 = dp("b3", [1], F32)

    # ---- per-core data ----
    idx_kv = dp("idx_kv", [P, IKV_W], I16)  # m-major ctx idx per gather group
    idx_type = dp("idx_type", [P, NL // 16], I16)
    idx_cat = dp("idx_cat", [P, NL // 16], I16)
    idx_uv = dp("idx_uv", [P, 2 * EL // 16], I16)
    logd = dp("logd", [1, NL], F32)
    kp = dp("kp", [NL, M], F32)                           # additive pad mask (0 / NEG)
    efT = dp("efT", [2, EL], BF16)

    out_d = nc.declare_dram_parameter("out", [EL], F32, isOutput=True)

    # ---- internal DRAM (kvloc/kvall ping-pong so next layer's QKV overlaps
    # this layer's gathers; the AllGather runs in halves, node-half-major) ----
    kvloc2 = [nc.dram_tensor(f"kvloc{i}", [NL, 2 * H], BF16) for i in range(2)]
    xloc = nc.dram_tensor("xloc", [NL, H], BF16)
    kvall2 = [nc.dram_tensor(f"kvall{i}", [N, 2 * H], BF16, addr_space="Shared")
              for i in range(2)]
    xall = nc.dram_tensor("xall", [N, H], BF16, addr_space="Shared")

    groups = [list(range(NC))]
    Alu = mybir.AluOpType
    Act = mybir.ActivationFunctionType

    with tile.TileContext(nc) as tc, ExitStack() as ctx:
        const = ctx.enter_context(tc.tile_pool(name="const", bufs=1))
        xpool = ctx.enter_context(tc.tile_pool(name="xpool", bufs=1))

        # ---------------- constants into SBUF ----------------
        gather = nc.gpsimd.dma_gather
        reg_nl = nc.gpsimd.to_reg(NL)
        reg_e2 = nc.gpsimd.to_reg(EL // 2)
        reg_grp = {}
        for g in groups_g:
            slots = sum(m_ts[t] for t in g)
            if slots not in reg_grp:
                reg_grp[slots] = nc.gpsimd.to_reg(slots * P)

        ident = const.tile([P, P], BF16)
        make_identity(nc, ident)

        def bcast_row(dram_ap, n, name):
            t = const.tile([P, n], F32, tag=name, name=name)
            src = bass.AP(tensor=dram_ap.tensor, offset=dram_ap.offset,
                          ap=[[0, P]] + dram_ap.ap)
            nc.sync.dma_start(out=t[:], in_=src)
            return t

        pb_b = bcast_row(proj_b[:], H, "pb")
        bq_b = [bcast_row(bq[ll, :], H, f"bq{ll}") for ll in range(L)]
        bo_b = [bcast_row(bo[ll, :], H, f"bo{ll}") for ll in range(L)]

        db_sb = const.tile([H // 4, 1], F32)
        nc.sync.dma_start(out=db_sb[:], in_=db.rearrange("(p o) -> p o", o=1))
        dw_sb = const.tile([1, H // 4], F32)
        nc.sync.dma_start(out=dw_sb[:], in_=dw[:])
        b1_sb = const.tile([P, 2], F32)
        nc.sync.dma_start(out=b1_sb[:], in_=b1[:])
        b2_sb = const.tile([H // 2, 1], F32)
        nc.sync.dma_start(out=b2_sb[:], in_=b2.rearrange("(p o) -> p o", o=1))
        b3_sb = const.tile([1, 1], F32)
        nc.sync.dma_start(out=b3_sb[:], in_=b3.rearrange("(p o) -> p o", o=1))

        ity_sb = const.tile([P, NL // 16], I16)
        nc.sync.dma_start(out=ity_sb[:], in_=idx_type[:])
        ica_sb = const.tile([P, NL // 16], I16)
        nc.sync.dma_start(out=ica_sb[:], in_=idx_cat[:])
        iuv_sb = const.tile([P, 2 * EL // 16], I16)
        nc.sync.dma_start(out=iuv_sb[:], in_=idx_uv[:])

        kp_sb = const.tile([P, NT, M], F32)
        nc.sync.dma_start(out=kp_sb[:], in_=kp.rearrange("(t p) m -> p t m", p=P))
        logd_sb = const.tile([1, NL], F32)
        nc.sync.dma_start(out=logd_sb[:], in_=logd[:])

        pw_sb = const.tile([P, 3, H], BF16)
        nc.sync.dma_start(out=pw_sb[:], in_=projWT.rearrange("c p o -> p c o"))
        w1_sb = const.tile([P, 4, H], BF16)
        nc.sync.dma_start(out=w1_sb[:], in_=w1T.rearrange("c p o -> p c o"))
        w1e_sb = const.tile([2, H], BF16)
        nc.sync.dma_start(out=w1e_sb[:], in_=w1eT[:])
        w2_sb = const.tile([P, 2, H // 2], BF16)
        nc.sync.dma_start(out=w2_sb[:], in_=w2T.rearrange("c p o -> p c o"))
        w3_sb = const.tile([P, 1], BF16)
        nc.sync.dma_start(out=w3_sb[:], in_=w3T[:])

        # all layers' in/out projection weights resident (1KB/partition each)
        wq_sb = const.tile([P, L, 2, H], BF16)
        nc.sync.dma_start(out=wq_sb[:], in_=wqT.rearrange("l c p o -> p l c o"))
        wk_sb = const.tile([P, L, 2, H], BF16)
        nc.sync.dma_start(out=wk_sb[:], in_=wkT.rearrange("l c p o -> p l c o"))
        wv_sb = const.tile([P, L, 2, H], BF16)
        nc.sync.dma_start(out=wv_sb[:], in_=wvT.rearrange("l c p o -> p l c o"))
        wo_sb = const.tile([P, L, 2, H], BF16)
        nc.sync.dma_start(out=wo_sb[:], in_=woT.rearrange("l c p o -> p l c o"))

        x_sb = xpool.tile([P, NT, H], BF16)

        work = ctx.enter_context(tc.tile_pool(name="work", bufs=1))
        gath = ctx.enter_context(tc.tile_pool(name="gath", bufs=2))
        att = ctx.enter_context(tc.tile_pool(name="att", bufs=2))
        psum = ctx.enter_context(tc.tile_pool(name="psum", bufs=2, space="PSUM"))
        psum1 = ctx.enter_context(tc.tile_pool(name="psum1", bufs=2, space="PSUM"))

        def qkv_tile(lq, g, q_t, kvloc_pv):
            """Project x tile g with layer-lq weights -> q_t[:, g], kvloc."""
            xT = work.tile([P, 2, P], BF16, tag="xT", name="xT", bufs=2)
            for c in range(2):
                pt = psum1.tile([P, P], BF16, tag="ptr", name="pt")
                nc.tensor.transpose(pt[:], x_sb[:, g, c * P:(c + 1) * P], ident[:])
                nc.scalar.activation(xT[:, c, :], pt[:], Act.Copy)
            pq = psum.tile([P, H], F32, tag="pqkv", name="pq")
            nc.tensor.matmul(pq[:], xT[:, 0, :], wq_sb[:, lq, 0, :], start=True, stop=False)
            nc.tensor.matmul(pq[:], xT[:, 1, :], wq_sb[:, lq, 1, :], start=False, stop=True)
            nc.vector.tensor_tensor(q_t[:, g, :], pq[:], bq_b[lq][:], op=Alu.add)
            pk = psum.tile([P, H], F32, tag="pqkv", name="pk")
            nc.tensor.matmul(pk[:], xT[:, 0, :], wk_sb[:, lq, 0, :], start=True, stop=False)
            nc.tensor.matmul(pk[:], xT[:, 1, :], wk_sb[:, lq, 1, :], start=False, stop=True)
            kev = work.tile([P, H], BF16, tag="kev", name="kev", bufs=2)
            nc.scalar.activation(kev[:], pk[:], Act.Copy)
            nc.sync.dma_start(out=kvloc_pv[:, g, 0:H], in_=kev[:])
            pv = psum.tile([P, H], F32, tag="pqkv", name="pv")
            nc.tensor.matmul(pv[:], xT[:, 0, :], wv_sb[:, lq, 0, :], start=True, stop=False)
            nc.tensor.matmul(pv[:], xT[:, 1, :], wv_sb[:, lq, 1, :], start=False, stop=True)
            vev = work.tile([P, H], BF16, tag="vev", name="vev", bufs=2)
            nc.scalar.activation(vev[:], pv[:], Act.Copy)
            nc.sync.dma_start(out=kvloc_pv[:, g, H:2 * H], in_=vev[:])

        def ag_q(src, dst, s):
            nsl = slice(s * (NL // 4), (s + 1) * (NL // 4))
            gsl = slice(s * (N // 4), (s + 1) * (N // 4))
            nc.gpsimd.collective_compute("AllGather", Alu.bypass, replica_groups=groups,
                                         ins=[src[nsl, :]], outs=[dst[gsl, :]])

        kvloc_pvs = [kv.rearrange("(t p) o -> p t o", p=P) for kv in kvloc2]
        xloc_pv = xloc.rearrange("(t p) o -> p t o", p=P)

        # ------------- node feature encoding + layer-0 QKV -------------
        with ExitStack() as ectx:
            enc = ectx.enter_context(tc.tile_pool(name="enc", bufs=1))
            teT = enc.tile([P, NL], BF16)
            gather(teT.rearrange("p (c n) -> p c n", c=1), type_tab[:],
                                 ity_sb[:], NL, reg_nl, H // 2, transpose=True, single_packet=False)
            ccT = enc.tile([P, NL], BF16)
            gather(ccT.rearrange("p (c n) -> p c n", c=1), cat_tab[:],
                                 ica_sb[:], NL, reg_nl, 2 * CD, transpose=True, single_packet=False)
            deT = enc.tile([P, NL], BF16)
            nc.vector.memset(deT[:], 0.0)
            for s in range(NL // 512):
                pd = psum.tile([H // 4, 512], F32, tag="pbig", name="pd")
                nc.tensor.matmul(pd[:], dw_sb[:], logd_sb[:, s * 512:(s + 1) * 512],
                                 start=True, stop=True)
                nc.scalar.activation(deT[0:H // 4, s * 512:(s + 1) * 512], pd[:],
                                     Act.Relu, bias=db_sb[:])
            q_cur = work.tile([P, NT, H], BF16, tag="q", name="q0", bufs=2)
            for g in reversed(range(NT)):
                px = psum.tile([P, H], F32, tag="pqkv", name="px")
                cs = slice(g * P, (g + 1) * P)
                nc.tensor.matmul(px[:], teT[:, cs], pw_sb[:, 0, :], start=True, stop=False)
                nc.tensor.matmul(px[:], ccT[:, cs], pw_sb[:, 1, :], start=False, stop=False)
                nc.tensor.matmul(px[:], deT[:, cs], pw_sb[:, 2, :], start=False, stop=True)
                nc.vector.tensor_tensor(x_sb[:, g, :], px[:], pb_b[:], op=Alu.add)
                qkv_tile(0, g, q_cur, kvloc_pvs[0])
                if g % 4 == 0:
                    ag_q(kvloc2[0], kvall2[0], g // 4)

        # ---------------- attention layers ----------------
        for ll in range(L):
            kvall_cur = kvall2[ll % 2]
            last = ll == L - 1
            if not last:
                q_nxt = work.tile([P, NT, H], BF16, tag="q", name=f"q{ll + 1}", bufs=2)
                kvloc_nxt, kvall_nxt = kvloc2[(ll + 1) % 2], kvall2[(ll + 1) % 2]
            for gi, grp in enumerate(groups_g):
                slots = sum(m_ts[t] for t in grp)
                iw = slots * P // 16
                ikv_t = gath.tile([P, M * P // 16], I16, tag="ikv", name="ikv_t")
                nc.sync.dma_start(out=ikv_t[:, 0:iw],
                                  in_=idx_kv[:, int(goff[gi]) * P // 16:
                                             int(goff[gi]) * P // 16 + iw])
                kvg = gath.tile([P, M, 2 * H], BF16, tag="kg", name="kvg")
                gather(kvg[:, 0:slots, :], kvall_cur[:], ikv_t[:, 0:iw],
                       slots * P, reg_grp[slots], 2 * H, single_packet=False)
                off = 0
                for t in grp:
                    mt = m_ts[t]
                    msl = slice(off, off + mt)
                    off += mt
                    kg = kvg[:, msl, 0:H]
                    vg = kvg[:, msl, H:2 * H]

                    # scores: s[n,m,h] = sum_d q*k  (d-tree, ping-pong pp<->ta)
                    pp = att.tile([P, M, HEADS, DH], BF16, tag="pp", name="pp")
                    qb = q_cur[:, t, None, :].to_broadcast([P, mt, H])
                    nc.vector.tensor_tensor(pp.rearrange("p m h d -> p m (h d)")[:, 0:mt],
                                            kg.rearrange("p m o -> p m o"), qb, op=Alu.mult)
                    ta = att.tile([P, M, HEADS, DH // 2], BF16, tag="ta", name="ta")
                    nc.vector.tensor_tensor(ta[:, 0:mt], pp[:, 0:mt, :, 0:32], pp[:, 0:mt, :, 32:64], op=Alu.add)
                    nc.vector.tensor_tensor(pp[:, 0:mt, :, 0:16], ta[:, 0:mt, :, 0:16], ta[:, 0:mt, :, 16:32], op=Alu.add)
                    nc.vector.tensor_tensor(ta[:, 0:mt, :, 0:8], pp[:, 0:mt, :, 0:8], pp[:, 0:mt, :, 8:16], op=Alu.add)
                    nc.vector.tensor_tensor(pp[:, 0:mt, :, 0:4], ta[:, 0:mt, :, 0:4], ta[:, 0:mt, :, 4:8], op=Alu.add)
                    nc.vector.tensor_tensor(ta[:, 0:mt, :, 0:2], pp[:, 0:mt, :, 0:2], pp[:, 0:mt, :, 2:4], op=Alu.add)
                    s_m = att.tile([P, M, HEADS], F32, tag="sm", name="s_m")
                    nc.vector.tensor_tensor(s_m[:, 0:mt], ta[:, 0:mt, :, 0], ta[:, 0:mt, :, 1], op=Alu.add)

                    kpb = kp_sb[:, t, 0:mt, None].to_broadcast([P, mt, HEADS])
                    nc.vector.tensor_tensor(s_m[:, 0:mt], s_m[:, 0:mt], kpb, op=Alu.add)
                    es = att.tile([P, M, HEADS], F32, tag="es", name="es")
                    nc.scalar.activation(es[:, 0:mt], s_m[:, 0:mt], Act.Exp)
                    sums = att.tile([P, HEADS], F32, tag="sums", name="sums")
                    if mt > 1:
                        nc.vector.tensor_reduce(sums[:], es[:, 0:mt].rearrange("p m h -> p h m"),
                                                axis=mybir.AxisListType.X, op=Alu.add)
                    else:
                        nc.scalar.activation(sums[:], es[:, 0, :], Act.Copy)
                    rs = att.tile([P, HEADS], F32, tag="rs", name="rs")
                    nc.vector.reciprocal(rs[:], sums[:])
                    attw = att.tile([P, M, HEADS], BF16, tag="attw", name="attw")
                    nc.vector.tensor_tensor(attw[:, 0:mt], es[:, 0:mt],
                                            rs[:, None, :].to_broadcast([P, mt, HEADS]), op=Alu.mult)

                    # AV: o[n,:] = sum_m attw * v  (ping-pong floor-halving m-tree)
                    av = att.tile([P, M, H], BF16, tag="pp", name="av")
                    nc.vector.tensor_tensor(av[:, 0:mt].rearrange("p m (h d) -> p m h d", h=HEADS),
                                            vg.rearrange("p m (h d) -> p m h d", h=HEADS),
                                            attw[:, 0:mt, :, None].to_broadcast([P, mt, HEADS, DH]),
                                            op=Alu.mult)
                    tm = att.tile([P, M // 2, H], BF16, tag="ta", name="tm")
                    o_sb = att.tile([P, H], BF16, tag="o", name="o_sb")
                    src, dst, m = av, tm, mt
                    if m == 1:
                        nc.scalar.activation(o_sb[:], av[:, 0, :], Act.Copy)
                    while m > 1:
                        if m % 2 == 1:
                            nc.vector.tensor_tensor(src[:, 0, :], src[:, 0, :], src[:, m - 1, :], op=Alu.add)
                            m -= 1
                        h2 = m // 2
                        dap = o_sb[:, None, :] if h2 == 1 else dst[:, 0:h2, :]
                        nc.vector.tensor_tensor(dap, src[:, 0:h2, :], src[:, h2:2 * h2, :], op=Alu.add)
                        src, dst = dst, src
                        m = h2

                    # out-proj + relu -> x
                    oT = att.tile([P, 2, P], BF16, tag="oT", name="oT")
                    for c in range(2):
                        pt = psum1.tile([P, P], BF16, tag="ptr", name="pt")
                        nc.tensor.transpose(pt[:], o_sb[:, c * P:(c + 1) * P], ident[:])
                        nc.scalar.activation(oT[:, c, :], pt[:], Act.Copy)
                    pxn = psum.tile([P, H], F32, tag="pmm", name="pxn")
                    nc.tensor.matmul(pxn[:], oT[:, 0, :], wo_sb[:, ll, 0, :], start=True, stop=False)
                    nc.tensor.matmul(pxn[:], oT[:, 1, :], wo_sb[:, ll, 1, :], start=False, stop=True)
                    nc.vector.tensor_tensor(x_sb[:, t, :], pxn[:], bo_b[ll][:], op=Alu.add)
                    nc.vector.tensor_scalar_max(x_sb[:, t, :], x_sb[:, t, :], 0.0)

                    # next layer's projections / final x store, overlapped with gathers
                    if not last:
                        qkv_tile(ll + 1, t, q_nxt, kvloc_pvs[(ll + 1) % 2])
                        if t % 4 == 0:
                            ag_q(kvloc_nxt, kvall_nxt, t // 4)
                    else:
                        nc.sync.dma_start(out=xloc_pv[:, t, :], in_=x_sb[:, t, :])
                        if t % 4 == 0:
                            ag_q(xloc, xall, t // 4)
            if not last:
                q_cur = q_nxt

        EQ = EL // 4
        for quar in range(4):
            hsl = slice(quar * (2 * EQ // 16), (quar + 1) * (2 * EQ // 16))
            uvg = gath.tile([P, 2, 2 * EQ], BF16, tag="kg", name="uvg")
            gather(uvg[:], xall[:], iuv_sb[:, hsl], 2 * EQ, reg_e2, H,
                                 transpose=True, single_packet=False)
            ug = uvg[:, :, 0:EQ]
            vg2 = uvg[:, :, EQ:2 * EQ]
            for e in range(EQ // 512):
                eg = quar * (EQ // 512) + e
                esl = slice(e * 512, (e + 1) * 512)
                ef_sb = att.tile([2, 512], BF16, tag="ef", name="ef_sb")
                nc.sync.dma_start(out=ef_sb[:], in_=efT[:, eg * 512:(eg + 1) * 512])
                h1T = att.tile([P, 2, 512], BF16, tag="h1T", name="h1T")
                for oc in range(2):
                    ph = psum.tile([P, 512], F32, tag="pbig", name="ph")
                    ocs = slice(oc * P, (oc + 1) * P)
                    nc.tensor.matmul(ph[:], w1_sb[:, 0, ocs], ug[:, 0, esl], start=True, stop=False)
                    nc.tensor.matmul(ph[:], w1_sb[:, 1, ocs], ug[:, 1, esl], start=False, stop=False)
                    nc.tensor.matmul(ph[:], w1_sb[:, 2, ocs], vg2[:, 0, esl], start=False, stop=False)
                    nc.tensor.matmul(ph[:], w1_sb[:, 3, ocs], vg2[:, 1, esl], start=False, stop=False)
                    nc.tensor.matmul(ph[:], w1e_sb[:, ocs], ef_sb[:], start=False, stop=True)
                    nc.scalar.activation(h1T[:, oc, :], ph[:], Act.Relu, bias=b1_sb[:, oc:oc + 1])
                ph2 = psum.tile([P, 512], F32, tag="pbig", name="ph2")
                nc.tensor.matmul(ph2[0:H // 2, :], w2_sb[:, 0, :], h1T[:, 0, :], start=True, stop=False)
                nc.tensor.matmul(ph2[0:H // 2, :], w2_sb[:, 1, :], h1T[:, 1, :], start=False, stop=True)
                h2T = att.tile([H // 2, 512], BF16, tag="h2T", name="h2T")
                nc.scalar.activation(h2T[:], ph2[0:H // 2, :], Act.Relu, bias=b2_sb[:])
                pl = psum.tile([1, 512], F32, tag="pmm", name="pl")
                nc.tensor.matmul(pl[:], w3_sb[:, :], h2T[:], start=True, stop=True)
                lo = att.tile([1, 512], F32, tag="lo", name="lo")
                nc.scalar.activation(lo[:], pl[:], Act.Identity, bias=b3_sb[:])
                nc.sync.dma_start(out=out_d.rearrange("(a b) -> a b", a=ET)[eg, None, :], in_=lo[:])

    nc.finalize()
    return nc


# --------------------------------------------------------------------------
# Host-side prep + runner
# --------------------------------------------------------------------------

_CACHE = {}


def _prep_maps(inputs, plan):
    f = {k: np.asarray(v) for k, v in inputs.items()}
    scale = 1.0 / np.sqrt(np.float32(DH))

    cat0, cat1 = f["cat_embed0"].astype(np.float32), f["cat_embed1"].astype(np.float32)
    cat_tab = np.zeros((V * V, 2 * CD), np.float32)
    for i0 in range(V):
        for i1 in range(V):
            cat_tab[i0 * V + i1] = np.concatenate([cat0[i0], cat1[i1]])

    projWT = np.zeros((3, P, H), np.float32)
    pwt = f["proj_w"].astype(np.float32).T  # [320, 256]
    projWT[0] = pwt[0:128]
    projWT[1] = pwt[128:256]
    projWT[2, 0:64] = pwt[256:320]

    wqT = np.empty((L, 2, P, H), np.float32)
    wkT = np.empty((L, 2, P, H), np.float32)
    wvT = np.empty((L, 2, P, H), np.float32)
    woT = np.empty((L, 2, P, H), np.float32)
    bq = np.empty((L, H), np.float32)
    bo = np.empty((L, H), np.float32)
    for ll in range(L):
        w = f["in_proj_w"][ll].astype(np.float32)
        b = f["in_proj_b"][ll].astype(np.float32)
        wq, wk, wv = w[0:H], w[H:2 * H], w[2 * H:3 * H]
        bq[ll] = b[0:H] * scale
        bv = b[2 * H:3 * H]
        for c in range(2):
            wqT[ll, c] = (wq * scale).T[c * P:(c + 1) * P]
            wkT[ll, c] = wk.T[c * P:(c + 1) * P]
            wvT[ll, c] = wv.T[c * P:(c + 1) * P]
            woT[ll, c] = f["out_w"][ll].astype(np.float32).T[c * P:(c + 1) * P]
        bo[ll] = f["out_b"][ll].astype(np.float32) + f["out_w"][ll].astype(np.float32) @ bv

    w1 = f["mlp_w1"].astype(np.float32)      # [256, 514]
    w1T_full = w1.T                           # [514, 256]
    w1T = np.stack([w1T_full[c * P:(c + 1) * P] for c in range(4)])
    w1eT = w1T_full[512:514]
    b1 = f["mlp_b1"].astype(np.float32).reshape(2, P).T  # [128, 2]
    w2T = np.stack([f["mlp_w2"].astype(np.float32).T[c * P:(c + 1) * P] for c in range(2)])
    w3T = f["mlp_w3"].astype(np.float32).T   # [128, 1]

    shared = {
        "type_tab": _bf(f["type_embed"]),
        "cat_tab": _bf(cat_tab),
        "dw": _f32(f["degree_w"].reshape(1, -1)),
        "db": _f32(f["degree_b"]),
        "projWT": _bf(projWT),
        "proj_b": _f32(f["proj_b"]),
        "wqT": _bf(wqT), "bq": _f32(bq),
        "wkT": _bf(wkT), "wvT": _bf(wvT),
        "woT": _bf(woT), "bo": _f32(bo),
        "w1T": _bf(w1T), "w1eT": _bf(w1eT), "b1": _f32(b1),
        "w2T": _bf(w2T), "b2": _f32(f["mlp_b2"]),
        "w3T": _bf(w3T), "b3": _f32(f["mlp_b3"]),
    }

    ctx = f["context_indices"].astype(np.int64)
    kpm = f["key_padding_mask"].astype(bool)
    order, m_ts = plan
    groups = _mk_groups(m_ts)

    # rank r -> core r % NC, position r // NC (striped so every core sees the
    # same per-tile context-length profile); kvall row ids are quarter-major
    # so the AllGather can run in four independent pieces.
    rank = np.empty(N, np.int64)
    rank[order] = np.arange(N)
    rc, rq = rank % NC, rank // NC
    NQ = NL // 4
    seg = rq // NQ
    new_row = seg * (N // 4) + rc * NQ + (rq - seg * NQ)

    maps = []
    for c in range(NC):
        nodes_c = order[c::NC]           # global ids of this core's nodes
        es = slice(c * EL, (c + 1) * EL)
        ctx_c = new_row[ctx[nodes_c]]    # [2048, 32]
        idx_kv = np.concatenate(
            [ctx_c[t * P:(t + 1) * P, 0:m_ts[t]].T.flatten()
             for grp in groups for t in grp])
        m = dict(shared)
        m["idx_kv"] = _wrap16(idx_kv)
        m["idx_type"] = _wrap16(f["type_idx"][nodes_c])
        m["idx_cat"] = _wrap16(f["cat_idx"][nodes_c, 0] * V + f["cat_idx"][nodes_c, 1])
        u_c, v_c = new_row[f["u_idx"][es]], new_row[f["v_idx"][es]]
        EQ = EL // 4
        m["idx_uv"] = _wrap16(np.concatenate(
            [np.concatenate([u_c[q * EQ:(q + 1) * EQ], v_c[q * EQ:(q + 1) * EQ]])
             for q in range(4)]))
        m["logd"] = _f32(f["log_degree"][nodes_c].reshape(1, NL))
        m["kp"] = _f32(np.where(kpm[nodes_c], NEG, 0.0))
        m["efT"] = _bf(f["edge_feats"][es].T)
        maps.append(m)
    return maps


def _plan(inputs):
    """Sort nodes by effective context length (descending), striped across
    cores; per-tile gather width m_ts[t] = max length in tile t (uniform
    across cores by construction)."""
    kpm = np.asarray(inputs["key_padding_mask"]).astype(bool)
    valid = ~kpm
    eff = np.where(valid.any(1), M - valid[:, ::-1].argmax(1), 1).astype(np.int64)
    order = np.argsort(-eff, kind="stable")
    eff_sorted = eff[order]
    m_ts = tuple(int(eff_sorted[t * P * NC]) for t in range(NT))
    return order, m_ts


def kernel(**inputs):
    plan = _plan(inputs)
    m_ts = plan[1]
    if m_ts not in _CACHE:
        _CACHE[m_ts] = build_program(m_ts)
    nc = _CACHE[m_ts]
    maps = _prep_maps(inputs, plan)
    res = run_bass_kernel_spmd(nc, maps, core_ids=list(range(NC)))
    return np.concatenate([res.results[c]["out"] for c in range(NC)]).astype(np.float32)


if __name__ == "__main__":
    nc = build_program([M] * NT)
    print("program built OK")

